# revision 1
# baseline (speedup 1.0000x reference)
"""Trainium2 Bass kernel for nn_BasicBlock (per-sample dynamic 3x3 convs +
sync-BN + residual ReLU), data-parallel over batch on 8 NeuronCores.

Reference semantics (B=16, C=64, H=W=128):
    out = relu(bn2(conv2(relu(bn1(conv1(x, f1))), f2)) + x)
with training-mode BN over full-batch (N,H,W) statistics.

Sharding: 2 samples per core. Per-sample convs become block-diagonal
128x128 matmuls (partitions 0-63 = sample A channels, 64-127 = sample B).
BN batch statistics are made exact via a tiny AllReduce of per-channel
(mean, var, mean^2) over the 16 (sample, core) groups.

Convs run as 9 shifted-tap matmuls per spatial tile ([128,512] PSUM
accumulation) against a zero-padded image held in SBUF. Matmul inputs are
bf16 (fast weight load; ~1 cycle/row streaming); accumulation is fp32.
Raw conv outputs are staged bf16 for BN stats, normalization math is fp32.
"""
import numpy as np

import concourse.bass as bass
import concourse.mybir as mybir
import concourse.tile as tile
from concourse import bacc
from concourse.bass_utils import run_bass_kernel_spmd
N_CORES = 8
B, C, H, W = 16, 64, 128, 128
SPC = B // N_CORES            # samples per core (2)
HP, WP = H + 2, W + 2         # padded image
TR = 4                        # image rows per spatial tile
NT = H // TR                  # 32 tiles
N = TR * W                    # 512 moving elements per matmul
NGROUPS = B                   # 16 (sample, core) stat groups of H*W each
BN_EPS = 1e-5

F32 = mybir.dt.float32
F32R = mybir.dt.float32r
BF16 = mybir.dt.bfloat16
USE_BF16_MM = True          # bf16 matmul inputs (fast weight load) vs f32r
MMDT = BF16 if USE_BF16_MM else F32R
AF = mybir.ActivationFunctionType
ALU = mybir.AluOpType

_CACHE = {}


def _build():
    nc = bacc.Bacc("TRN2", target_bir_lowering=False, debug=False,
                   num_devices=N_CORES)
    xp_ext = nc.dram_tensor("xp", [128, HP, WP], MMDT, kind="ExternalInput").ap()
    w_ext = nc.dram_tensor("w", [128, 2, 9, 128], MMDT, kind="ExternalInput").ap()
    cst_ext = nc.dram_tensor("cst", [128, 4], F32, kind="ExternalInput").ap()
    z_ext = nc.dram_tensor("z", [128, WP], MMDT, kind="ExternalInput").ap()
    out_ext = nc.dram_tensor("out", [128, H, W], F32, kind="ExternalOutput").ap()

    dma_engines = [nc.sync, nc.gpsimd, nc.scalar]

    with tile.TileContext(nc) as tc:
        with tc.tile_pool(name="sb", bufs=1) as sb, \
             tc.tile_pool(name="ps", bufs=8, space="PSUM") as ps, \
             tc.tile_pool(name="fin", bufs=8) as fin, \
             tc.tile_pool(name="dram", bufs=1, space="DRAM") as dram:

            x_pad = sb.tile([128, HP * WP], MMDT, tag="x_pad")
            norm_pad = sb.tile([128, HP * WP], MMDT, tag="norm_pad")
            raw = sb.tile([128, H * W], BF16, tag="raw")
            wsb = sb.tile([128, 2 * 9 * 128], MMDT, tag="wsb")
            cst = sb.tile([128, 4], F32, tag="cst")
            st6 = [sb.tile([128, NT * 6], F32, tag=f"st6_{c}", name=f"st6_{c}")
                   for c in range(2)]
            gst = sb.tile([128, 3 * 2], F32, tag="gst")
            params = sb.tile([128, 4], F32, tag="params")   # a1 b1 a2 b2
            sml = sb.tile([128, 16], F32, tag="sml")        # small scratch

            cc_in = dram.tile([128 * 3], F32)
            cc_out = dram.tile([128 * 3], F32)
            warm_in = dram.tile([8], F32)
            warm_out = dram.tile([8], F32)

            x3 = x_pad.rearrange("p (h w) -> p h w", h=HP)
            n3 = norm_pad.rearrange("p (h w) -> p h w", h=HP)
            wv = wsb.rearrange("p (c t m) -> p c t m", c=2, t=9)

            # ---- phase 0: warmup collective + input DMAs ----
            nc.gpsimd.collective_compute(
                "AllReduce", ALU.add,
                replica_groups=[list(range(N_CORES))],
                ins=[warm_in.opt()], outs=[warm_out.opt()])

            nc.scalar.dma_start(out=wsb[:, :],
                                in_=w_ext.rearrange("k c t m -> k (c t m)"))
            nc.scalar.dma_start(out=cst[:, :], in_=cst_ext)

            # norm_pad borders <- zeros (conv2's padding)
            zsrc = z_ext
            nc.gpsimd.dma_start(out=n3[:, 0, :], in_=zsrc)
            nc.gpsimd.dma_start(out=n3[:, HP - 1, :], in_=zsrc)
            nc.sync.dma_start(out=n3[:, :, 0], in_=zsrc[:, 0:HP])
            nc.sync.dma_start(out=n3[:, :, WP - 1], in_=zsrc[:, 0:HP])

            # x (pre-padded on host) in 8 chunks round-robin across queues
            xin = xp_ext
            bounds = [0, 4, 8, 12, 16, 20, 26, 32, 40, 50, 62, 76, 92, 110, HP]
            for ch in range(len(bounds) - 1):
                r0, r1 = bounds[ch], bounds[ch + 1]
                eng = dma_engines[ch % 3]
                eng.dma_start(out=x3[:, r0:r1, :], in_=xin[:, r0:r1, :])

            # ---- conv + stats helper ----
            def conv_phase(src3, conv_idx, st6_t):
                for t in range(NT):
                    psum = ps.tile([128, N], F32, tag="psum")
                    r0 = t * TR
                    for tap in range(9):
                        kh, kw = tap // 3, tap % 3
                        rhs = src3[:, r0 + kh:r0 + kh + TR, kw:kw + W]
                        nc.tensor.matmul(psum[:, :], wv[:, conv_idx, tap, :], rhs,
                                         start=(tap == 0), stop=(tap == 8))
                    rt = raw[:, t * N:(t + 1) * N]
                    nc.scalar.activation(rt, psum[:, :], AF.Copy)
                    nc.vector.bn_stats(st6_t[:, t * 6:(t + 1) * 6], psum[:, :])

            # ---- BN stats -> per-channel scale/bias (exact sync-BN) ----
            def bn_params(st6_t, gamma_ap, beta_ap, a_ap, b_ap):
                s3 = sml[:, 4:7]
                mv = s3[:, 0:2]
                nc.vector.bn_aggr(mv, st6_t.rearrange("p (t k) -> p t k", k=6))
                nc.vector.tensor_mul(s3[:, 2:3], mv[:, 0:1], mv[:, 0:1])
                nc.sync.dma_start(out=cc_in[:], in_=s3)
                nc.gpsimd.collective_compute(
                    "AllReduce", ALU.add,
                    replica_groups=[list(range(N_CORES))],
                    ins=[cc_in.opt()], outs=[cc_out.opt()])
                # bring back both sample-halves: dest [p, k, s], k=3 stats
                src = cc_out.rearrange("(s c k) -> c k s", s=2, k=3)
                nc.sync.dma_start(out=gst.rearrange("p (k s) -> p k s", k=3)[0:64],
                                  in_=src)
                nc.gpsimd.dma_start(out=gst.rearrange("p (k s) -> p k s", k=3)[64:128],
                                    in_=src)
                gsum = sml[:, 8:11]
                nc.vector.tensor_reduce(gsum, gst.rearrange("p (k s) -> p k s", k=3),
                                        axis=mybir.AxisListType.X, op=ALU.add)
                nc.vector.tensor_scalar_mul(gsum, gsum, 1.0 / NGROUPS)
                mean_g = gsum[:, 0:1]
                m2g = sml[:, 11:12]
                nc.vector.tensor_mul(m2g, mean_g, mean_g)
                v = sml[:, 12:13]                              # var (eps via sqrt bias)
                nc.vector.scalar_tensor_tensor(v, m2g, -1.0, gsum[:, 1:2],
                                               op0=ALU.mult, op1=ALU.add)
                nc.vector.tensor_add(v, v, gsum[:, 2:3])
                ve = sml[:, 13:14]
                nc.vector.tensor_scalar_add(ve, v, BN_EPS)     # v + eps
                sd = sml[:, 14:15]
                nc.scalar.activation(sd, ve, AF.Sqrt)
                y0 = sml[:, 15:16]
                nc.vector.reciprocal(y0, sd)
                # one Newton step for rsqrt accuracy: y1 = y0*(1.5 - 0.5*ve*y0^2)
                tn = sml[:, 3:4]
                nc.vector.tensor_mul(tn, ve, y0)
                nc.vector.tensor_mul(tn, tn, y0)
                nc.vector.tensor_scalar(tn, tn, -0.5, 1.5, op0=ALU.mult, op1=ALU.add)
                nc.vector.tensor_mul(y0, y0, tn)
                nc.vector.tensor_mul(a_ap, y0, gamma_ap)
                nc.vector.tensor_mul(tn, mean_g, a_ap)
                nc.vector.tensor_sub(b_ap, beta_ap, tn)

            # ---- pipeline ----
            conv_phase(x3, 0, st6[0])
            bn_params(st6[0], cst[:, 0:1], cst[:, 1:2], params[:, 0:1], params[:, 1:2])

            # norm1: relu(a1*raw + b1) -> norm_pad interior. Interleaved with
            # conv2 emission (3 tiles ahead) so conv2's PSUM evacuations are
            # not queued behind the whole norm1 backlog on ACT's strict FIFO.
            def norm1_tile(t):
                rt = raw[:, t * N:(t + 1) * N].rearrange("p (a b) -> p a b", a=TR)
                dst = n3[:, 1 + t * TR:1 + (t + 1) * TR, 1:1 + W]
                nc.scalar.activation(dst, rt, AF.Relu,
                                     scale=params[:, 0:1], bias=params[:, 1:2])

            for t in range(3):
                norm1_tile(t)
            for t in range(NT):
                if t + 3 < NT:
                    norm1_tile(t + 3)
                psum = ps.tile([128, N], F32, tag="psum", name=f"psum2_{t}")
                r0 = t * TR
                for tap in range(9):
                    kh, kw = tap // 3, tap % 3
                    rhs = n3[:, r0 + kh:r0 + kh + TR, kw:kw + W]
                    nc.tensor.matmul(psum[:, :], wv[:, 1, tap, :], rhs,
                                     start=(tap == 0), stop=(tap == 8))
                rt2 = raw[:, t * N:(t + 1) * N]
                nc.scalar.activation(rt2, psum[:, :], AF.Copy)
                nc.vector.bn_stats(st6[1][:, t * 6:(t + 1) * 6], psum[:, :])
            bn_params(st6[1], cst[:, 2:3], cst[:, 3:4], params[:, 2:3], params[:, 3:4])

            # final: relu(a2*raw2 + b2 + x) -> DMA out
            # (a2*raw2 + x) on DVE, then (+b2, relu) fused on ACT
            for t in range(NT):
                rt = raw[:, t * N:(t + 1) * N].rearrange("p (a b) -> p a b", a=TR)
                xt = x3[:, 1 + t * TR:1 + (t + 1) * TR, 1:1 + W]
                if not USE_BF16_MM:
                    xt = xt.bitcast(F32)
                ft = fin.tile([128, TR, W], F32, tag="fin")
                nc.vector.scalar_tensor_tensor(ft[:, :, :], rt, params[:, 2:3], xt,
                                             op0=ALU.mult, op1=ALU.add)
                nc.scalar.activation(ft[:, :, :], ft[:, :, :], AF.Relu,
                                     bias=params[:, 3:4])
                nc.sync.dma_start(out=out_ext[:, t * TR:(t + 1) * TR, :],
                                  in_=ft[:, :, :])

    nc.compile()
    return nc


def _get_nc():
    if "nc" not in _CACHE:
        _CACHE["nc"] = _build()
    return _CACHE["nc"]


def _pack_inputs(x, filters1, filters2, gamma1, beta1, gamma2, beta2):
    import ml_dtypes
    mmdt = ml_dtypes.bfloat16 if USE_BF16_MM else np.float32
    x = np.ascontiguousarray(x, dtype=np.float32)
    in_maps = []
    gb = np.stack([np.tile(np.asarray(g, np.float32), 2) for g in
                   (gamma1, beta1, gamma2, beta2)], axis=1)  # [128, 4]
    z = np.zeros((128, WP), mmdt)
    for i in range(N_CORES):
        s0, s1 = SPC * i, SPC * i + 1
        xp = np.zeros((128, HP, WP), mmdt)
        xp[0:C, 1:1 + H, 1:1 + W] = x[s0]
        xp[C:128, 1:1 + H, 1:1 + W] = x[s1]
        w = np.zeros((128, 2, 9, 128), mmdt)
        for ci, f in enumerate((filters1, filters2)):
            f = np.asarray(f, np.float32)
            # w[k, ci, tap, m]: lhsT[k=cin, m=cout], block-diagonal over samples
            fs0 = f[s0].transpose(1, 2, 3, 0).reshape(C, 9, C)   # [cin, tap, cout]
            fs1 = f[s1].transpose(1, 2, 3, 0).reshape(C, 9, C)
            w[0:C, ci, :, 0:C] = fs0
            w[C:128, ci, :, C:128] = fs1
        in_maps.append({"xp": xp, "w": w, "cst": gb, "z": z})
    return in_maps


def _run(in_maps, trace=False):
    nc = _get_nc()
    return run_bass_kernel_spmd(nc, in_maps, core_ids=list(range(N_CORES)),
                                trace=trace)


def kernel(x, filters1, filters2, gamma1, beta1, gamma2, beta2):
    in_maps = _pack_inputs(x, filters1, filters2, gamma1, beta1, gamma2, beta2)
    res = _run(in_maps, trace=False)
    out = np.empty((B, C, H, W), np.float32)
    for i in range(N_CORES):
        o = res.results[i]["out"]
        out[SPC * i] = o[0:C]
        out[SPC * i + 1] = o[C:128]
    return out

